# revision 24
# baseline (speedup 1.0000x reference)
"""MincutPool Trainium2 kernel.

Per-core (data-parallel over batch, 1 batch element per NeuronCore):
  S = softmax(relu(X @ W_in + b_in) @ W_out + b_out)        (N, KC)
  C' = [S | ssum]^T @ A   (KC+1, N), ssum[n] = sum_k S[n,k]^2
  A_pooled = C'[:KC] @ S  (KC, KC)
  X_pooled = S^T @ X      (KC, F)
  den = sum_m C'[KC, m]   (= tr(S^T D S), no row-sums of A materialized)
Host: losses + A_norm normalization on the tiny (KC,KC) outputs.

The single pass over A streams row-tiles (partition = n) so no transposed
loads of A are ever needed; the only transposes are X-tiles (128x128) and
C' chunks (32x128), both done on the tensor engine.
"""

import os
import numpy as np
from contextlib import ExitStack

import concourse.bass as bass
import concourse.bacc as bacc
import concourse.tile as tile
from concourse import mybir
from concourse.bass_utils import run_bass_kernel_spmd

AF = mybir.ActivationFunctionType
ALU = mybir.AluOpType
F32 = mybir.dt.float32
F32R = mybir.dt.float32r

B, N, F, H, KC = 8, 3000, 128, 256, 32
EPS = 1e-7
P = 128      # partition tile (rows of A / X per tile)
MMF = 512    # max moving free dim for fp32 matmul (one PSUM bank)


def build_nc(n=N, use_f32r=True, stage=3, loop_reps=1,
             round_engine="vector", adt="f32r", abufs=3, rbufs=2, wbufs=3):
    nt = (n + P - 1) // P          # row tiles
    nch = (n + MMF - 1) // MMF     # column chunks of C'
    KA = KC + 1                    # S augmented with the ssum column
    BF16 = mybir.dt.bfloat16
    if adt == "bf16":
        MMDT = BF16
    elif adt == "f32":
        MMDT = F32
        use_f32r = False
    else:
        MMDT = F32R
    use_round = adt != "f32"

    nc = bacc.Bacc(None, target_bir_lowering=False)
    A_ext = nc.declare_dram_parameter("A", [n, n], F32, isOutput=False)
    X_ext = nc.declare_dram_parameter("X", [n, F], F32, isOutput=False)
    Wi_ext = nc.declare_dram_parameter("W_in", [F, H], F32, isOutput=False)
    bi_ext = nc.declare_dram_parameter("b_in", [H, 1], F32, isOutput=False)
    Wo_ext = nc.declare_dram_parameter("W_out", [H, KC], F32, isOutput=False)
    bo_ext = nc.declare_dram_parameter("b_out", [1, KC], F32, isOutput=False)
    id_ext = nc.declare_dram_parameter("ident", [P, P], F32, isOutput=False)
    S_ext = nc.declare_dram_parameter("S_out", [n, KC], F32, isOutput=True)
    XP_ext = nc.declare_dram_parameter("XP", [KC, F], F32, isOutput=True)
    APo_ext = nc.declare_dram_parameter("AP_raw", [KC, KC], F32, isOutput=True)
    den_ext = nc.declare_dram_parameter("den", [1, 1], F32, isOutput=True)

    with TileCtx(nc) as tc, ExitStack() as octx:
        if loop_reps > 1:
            octx.enter_context(tc.For_i(0, loop_reps, 1))
        ctx = octx.enter_context(ExitStack())
        consts = ctx.enter_context(tc.tile_pool(name="consts", bufs=1))
        xpool = ctx.enter_context(tc.tile_pool(name="xpool", bufs=nt))
        spool = ctx.enter_context(tc.tile_pool(name="spool", bufs=nt))
        apool = ctx.enter_context(tc.tile_pool(name="apool", bufs=abufs))
        work = ctx.enter_context(tc.tile_pool(name="work", bufs=wbufs))
        csb = ctx.enter_context(tc.tile_pool(name="csb", bufs=1))
        outp = ctx.enter_context(tc.tile_pool(name="outp", bufs=1))

        # ---- constants ----
        wi_s = consts.tile([F, H], F32)
        nc.sync.dma_start(out=wi_s, in_=Wi_ext[:, :])
        wo0_s = consts.tile([P, KC], F32)
        nc.sync.dma_start(out=wo0_s, in_=Wo_ext[0:P, :])
        wo1_s = consts.tile([P, KC], F32)
        nc.sync.dma_start(out=wo1_s, in_=Wo_ext[P:H, :])
        bi0_s = consts.tile([P, 1], F32)
        nc.sync.dma_start(out=bi0_s, in_=bi_ext[0:P, :])
        bi1_s = consts.tile([P, 1], F32)
        nc.sync.dma_start(out=bi1_s, in_=bi_ext[P:H, :])
        bo_s = consts.tile([P, KC], F32)
        nc.sync.dma_start(out=bo_s, in_=bo_ext[0:1, :].partition_broadcast(P))
        id_s = consts.tile([P, P], F32)
        nc.sync.dma_start(out=id_s, in_=id_ext[:, :])

        x_tiles = []
        s_tiles = []
        row_cnt = []

        # ---- main pass: S per tile + stream A once ----
        with ExitStack() as mctx:
            cpool = mctx.enter_context(
                tc.tile_pool(name="cpool", bufs=1, space="PSUM"))
            wp = mctx.enter_context(
                tc.tile_pool(name="wp", bufs=2, space="PSUM"))

            cp_tiles = []
            for c in range(nch):
                w = min(MMF, n - c * MMF)
                cpt = cpool.tile([KA, w], F32, name=f"cp{c}", tag=f"cp{c}")
                cp_tiles.append(cpt)

            for t in range(nt):
                r0 = t * P
                rows = min(P, n - r0)
                x_t = xpool.tile([P, F], F32, name="x_t", tag="x_t")
                nc.sync.dma_start(out=x_t[:rows, :], in_=X_ext[r0:r0 + rows, :])
                a_t = apool.tile([P, n], F32, name="a_t", tag="a_t")
                nc.sync.dma_start(out=a_t[:rows, :], in_=A_ext[r0:r0 + rows, :])
                # rounded fp32r/bf16 copy of the A tile (PE full-rate matmul)
                if use_round:
                    a_r = apool.tile([P, n], MMDT, name="a_r", tag="a_r",
                                     bufs=rbufs)
                    eng = nc.gpsimd if round_engine == "gpsimd" else nc.vector
                    eng.tensor_copy(out=a_r[:rows, :], in_=a_t[:rows, :])
                else:
                    a_r = a_t

                # X_t^T via PE transpose
                xt_p = wp.tile([P, P], F32, name="xt_p", tag="w")
                nc.tensor.transpose(
                    out=xt_p[:, :rows], in_=x_t[:rows, :],
                    identity=id_s[:rows, :rows])
                xt_s = work.tile([P, P], F32, name="xt_s", tag="xt_s")
                nc.vector.tensor_copy(out=xt_s[:, :rows], in_=xt_p[:, :rows])

                # Hid^T = relu(W_in^T X^T + b_in), h-partitioned in 2 chunks
                hidT = []
                for c in range(2):
                    hid_p = wp.tile([P, P], F32, name="hid_p", tag="w")
                    nc.tensor.matmul(
                        out=hid_p[:, :rows],
                        lhsT=wi_s[:, c * P:(c + 1) * P],
                        rhs=xt_s[:, :rows],
                        start=True, stop=True)
                    hid_s = work.tile([P, P], F32, name="hid_s", tag=f"hid{c}")
                    bi = bi0_s if c == 0 else bi1_s
                    # relu(x + b) fused on DVE, PSUM -> SBUF
                    nc.vector.tensor_scalar(
                        out=hid_s[:, :rows], in0=hid_p[:, :rows],
                        scalar1=bi, scalar2=0.0,
                        op0=ALU.add, op1=ALU.max)
                    hidT.append(hid_s)

                # logits (rows, KC)
                lg_p = wp.tile([P, KC], F32, name="lg_p", tag="w")
                for c in range(2):
                    nc.tensor.matmul(
                        out=lg_p[:rows, :],
                        lhsT=hidT[c][:, :rows],
                        rhs=(wo0_s if c == 0 else wo1_s),
                        start=(c == 0), stop=(c == 1))
                lb = work.tile([P, KC], F32, name="lb", tag="lb")
                nc.vector.tensor_add(out=lb[:rows, :], in0=lg_p[:rows, :],
                                     in1=bo_s[:rows, :])

                # softmax over KC (free axis)
                s_t = spool.tile([P, KA], F32, name="s_t", tag="s_t")
                nmax = work.tile([P, 1], F32, name="nmax", tag="nmax")
                nc.vector.tensor_reduce(
                    out=nmax[:rows], in_=lb[:rows, :],
                    axis=mybir.AxisListType.X, op=ALU.max, negate=True)
                sexp = work.tile([P, 1], F32, name="sexp", tag="sexp")
                nc.scalar.activation(
                    out=s_t[:rows, 0:KC], in_=lb[:rows, :], func=AF.Exp,
                    bias=nmax[:rows], scale=1.0, accum_out=sexp[:rows])
                rcp = work.tile([P, 1], F32, name="rcp", tag="rcp")
                nc.vector.reciprocal(out=rcp[:rows], in_=sexp[:rows])
                nc.vector.tensor_scalar_mul(s_t[:rows, 0:KC], s_t[:rows, 0:KC],
                                            rcp[:rows])
                # ssum = sum_k S^2 -> augmented column
                # (tensor_tensor_reduce crashes TRN2 here; use mul+reduce)
                ssq = work.tile([P, KC], F32, name="ssq", tag="ssq")
                nc.vector.tensor_mul(ssq[:rows, :], s_t[:rows, 0:KC],
                                     s_t[:rows, 0:KC])
                nc.vector.tensor_reduce(
                    out=s_t[:rows, KC:KA], in_=ssq[:rows, :],
                    axis=mybir.AxisListType.X, op=ALU.add)

                nc.sync.dma_start(out=S_ext[r0:r0 + rows, :],
                                  in_=s_t[:rows, 0:KC])

                # rounded copy of Saug for the streaming matmuls
                if use_round:
                    s_r = spool.tile([P, KA], MMDT, name="s_r", tag="s_r")
                    nc.vector.tensor_copy(out=s_r[:rows, :], in_=s_t[:rows, :])
                else:
                    s_r = s_t

                # C' += Saug_t^T @ A_t   (the big streaming matmuls, fp32r)
                if stage >= 2:
                    for c in range(nch):
                        c0 = c * MMF
                        w = min(MMF, n - c0)
                        nc.tensor.matmul(
                            out=cp_tiles[c][:, :],
                            lhsT=s_r[:rows, :],
                            rhs=a_r[:rows, c0:c0 + w],
                            start=(t == 0), stop=(t == nt - 1),
                            skip_group_check=True)

                x_tiles.append(x_t)
                s_tiles.append(s_t)
                row_cnt.append(rows)

            # C' -> SBUF, den
            cs = csb.tile([KA, n], F32)
            den_s = outp.tile([1, 1], F32)
            if stage >= 2:
                for c in range(nch):
                    c0 = c * MMF
                    w = min(MMF, n - c0)
                    nc.vector.tensor_copy(out=cs[:, c0:c0 + w], in_=cp_tiles[c])
                nc.vector.tensor_reduce(
                    out=den_s, in_=cs[KC:KA, :],
                    axis=mybir.AxisListType.X, op=ALU.add)
            else:
                nc.vector.memset(cs, 0.0)
                nc.vector.memset(den_s, 1.0)
            nc.sync.dma_start(out=den_ext[:, :], in_=den_s)

        # ---- epilogue: A_pooled = C @ S, X_pooled = S^T X ----
        with ExitStack() as ectx:
            ep = ectx.enter_context(
                tc.tile_pool(name="ep", bufs=2, space="PSUM"))
            accp = ectx.enter_context(
                tc.tile_pool(name="accp", bufs=1, space="PSUM"))
            ap_p = accp.tile([KC, KC], F32, name="ap_p", tag="ap_p")
            xp_p = accp.tile([KC, F], F32, name="xp_p", tag="xp_p")

            for t in range(nt if stage >= 3 else 0):
                r0 = t * P
                rows = row_cnt[t]
                ct_p = ep.tile([P, KC], F32, name="ct_p", tag="ct")
                nc.tensor.transpose(
                    out=ct_p[:rows, :], in_=cs[0:KC, r0:r0 + rows],
                    identity=id_s[0:KC, 0:KC])
                ct_s = work.tile([P, KC], F32, name="ct_s", tag="ct_s")
                nc.vector.tensor_copy(out=ct_s[:rows, :], in_=ct_p[:rows, :])
                nc.tensor.matmul(
                    out=ap_p, lhsT=ct_s[:rows, :],
                    rhs=s_tiles[t][:rows, 0:KC],
                    start=(t == 0), stop=(t == nt - 1),
                    skip_group_check=True)
                nc.tensor.matmul(
                    out=xp_p, lhsT=s_tiles[t][:rows, 0:KC],
                    rhs=x_tiles[t][:rows, :],
                    start=(t == 0), stop=(t == nt - 1),
                    skip_group_check=True)

            ap_s = outp.tile([KC, KC], F32)
            xp_s = outp.tile([KC, F], F32)
            if stage >= 3:
                nc.vector.tensor_copy(out=ap_s, in_=ap_p)
                nc.vector.tensor_copy(out=xp_s, in_=xp_p)
            else:
                nc.vector.memset(ap_s, 0.0)
                nc.vector.memset(xp_s, 0.0)
            nc.sync.dma_start(out=APo_ext[:, :], in_=ap_s)
            nc.sync.dma_start(out=XP_ext[:, :], in_=xp_s)

    if not nc.is_finalized():
        nc.finalize()
    return nc


def TileCtx(nc):
    return tile.TileContext(nc)


_built = {}


def _get_nc(n=N):
    use_f32r = not bool(os.environ.get("MINCUT_NO_F32R"))
    stage = int(os.environ.get("MINCUT_STAGE", "3"))
    key = (n, use_f32r, stage)
    if key not in _built:
        _built[key] = build_nc(n, use_f32r=use_f32r, stage=stage)
    return _built[key]


LAST_RESULTS = None


def kernel(X, A, kernel_in, bias_in, kernel_out, bias_out):
    global LAST_RESULTS
    X = np.asarray(X, dtype=np.float32)
    A = np.asarray(A, dtype=np.float32)
    kernel_in = np.asarray(kernel_in, dtype=np.float32)
    bias_in = np.asarray(bias_in, dtype=np.float32)
    kernel_out = np.asarray(kernel_out, dtype=np.float32)
    bias_out = np.asarray(bias_out, dtype=np.float32)

    n = X.shape[1]
    nc = _get_nc(n)
    ident = np.eye(P, dtype=np.float32)
    in_maps = [{
        "A": np.ascontiguousarray(A[b]),
        "X": np.ascontiguousarray(X[b]),
        "W_in": kernel_in,
        "b_in": bias_in.reshape(H, 1),
        "W_out": kernel_out,
        "b_out": bias_out.reshape(1, KC),
        "ident": ident,
    } for b in range(B)]

    trace = bool(os.environ.get("MINCUT_TRACE"))
    LAST_RESULTS = run_bass_kernel_spmd(
        nc, in_maps, list(range(B)), trace=trace)
    res = LAST_RESULTS.results

    S = np.stack([res[b]["S_out"] for b in range(B)]).astype(np.float32)
    X_pooled = np.stack([res[b]["XP"] for b in range(B)]).astype(np.float32)
    APr = np.stack([res[b]["AP_raw"] for b in range(B)]).astype(np.float64)
    den = np.array([float(np.asarray(res[b]["den"]).reshape(-1)[0])
                    for b in range(B)])

    num = np.trace(APr, axis1=1, axis2=2)
    cut_loss = np.float32(np.mean(-(num / den)))
    ortho_loss = cut_loss

    kc = APr.shape[-1]
    Ap0 = APr * (1.0 - np.eye(kc))
    Dp = np.sqrt(Ap0.sum(-1)) + EPS          # (B, kc)
    A_norm = (Ap0 / Dp[:, None, :] / Dp[:, :, None]).astype(np.float32)

    return X_pooled, A_norm, S, cut_loss, ortho_loss


# revision 27
# speedup vs baseline: 2.6988x; 2.6988x over previous
"""MincutPool Trainium2 kernel.

Per-core (data-parallel over batch, 1 batch element per NeuronCore):
  S = softmax(relu(X @ W_in + b_in) @ W_out + b_out)        (N, KC)
  C' = [S | ssum]^T @ A   (KC+1, N), ssum[n] = sum_k S[n,k]^2
  A_pooled = C'[:KC] @ S  (KC, KC)
  X_pooled = S^T @ X      (KC, F)
  den = sum_m C'[KC, m]   (= tr(S^T D S), no row-sums of A materialized)
Host: losses + A_norm normalization on the tiny (KC,KC) outputs.

The single pass over A streams row-tiles (partition = n) so no transposed
loads of A are ever needed; the only transposes are X-tiles (128x128) and
C' chunks (32x128), both done on the tensor engine.
"""

import os
import numpy as np
from contextlib import ExitStack

import concourse.bass as bass
import concourse.bacc as bacc
import concourse.tile as tile
from concourse import mybir
from concourse.bass_utils import run_bass_kernel_spmd

AF = mybir.ActivationFunctionType
ALU = mybir.AluOpType
F32 = mybir.dt.float32
F32R = mybir.dt.float32r

B, N, F, H, KC = 8, 3000, 128, 256, 32
EPS = 1e-7
P = 128      # partition tile (rows of A / X per tile)
MMF = 512    # max moving free dim for fp32 matmul (one PSUM bank)


def build_nc(n=N, use_f32r=True, stage=3, loop_reps=1,
             round_engine="vector", adt="f32r", abufs=3, rbufs=2, wbufs=3):
    nt = (n + P - 1) // P          # row tiles
    nch = (n + MMF - 1) // MMF     # column chunks of C'
    KA = KC + 1                    # S augmented with the ssum column
    BF16 = mybir.dt.bfloat16
    if adt == "bf16":
        MMDT = BF16
    elif adt == "f32":
        MMDT = F32
        use_f32r = False
    else:
        MMDT = F32R
    use_round = adt != "f32"

    nc = bacc.Bacc(None, target_bir_lowering=False)
    A_ext = nc.declare_dram_parameter("A", [n, n], F32, isOutput=False)
    X_ext = nc.declare_dram_parameter("X", [n, F], F32, isOutput=False)
    Wi_ext = nc.declare_dram_parameter("W_in", [F, H], F32, isOutput=False)
    bi_ext = nc.declare_dram_parameter("b_in", [H, 1], F32, isOutput=False)
    Wo_ext = nc.declare_dram_parameter("W_out", [H, KC], F32, isOutput=False)
    bo_ext = nc.declare_dram_parameter("b_out", [1, KC], F32, isOutput=False)
    id_ext = nc.declare_dram_parameter("ident", [P, P], F32, isOutput=False)
    S_ext = nc.declare_dram_parameter("S_out", [n, KC], F32, isOutput=True)
    XP_ext = nc.declare_dram_parameter("XP", [KC, F], F32, isOutput=True)
    APo_ext = nc.declare_dram_parameter("AP_raw", [KC, KC], F32, isOutput=True)
    den_ext = nc.declare_dram_parameter("den", [1, 1], F32, isOutput=True)

    with TileCtx(nc) as tc, ExitStack() as octx:
        if loop_reps > 1:
            octx.enter_context(tc.For_i(0, loop_reps, 1))
        ctx = octx.enter_context(ExitStack())
        consts = ctx.enter_context(tc.tile_pool(name="consts", bufs=1))
        xpool = ctx.enter_context(tc.tile_pool(name="xpool", bufs=nt))
        spool = ctx.enter_context(tc.tile_pool(name="spool", bufs=nt))
        apool = ctx.enter_context(tc.tile_pool(name="apool", bufs=abufs))
        work = ctx.enter_context(tc.tile_pool(name="work", bufs=wbufs))
        csb = ctx.enter_context(tc.tile_pool(name="csb", bufs=1))
        outp = ctx.enter_context(tc.tile_pool(name="outp", bufs=1))

        # ---- constants ----
        wi_s = consts.tile([F, H], F32)
        nc.sync.dma_start(out=wi_s, in_=Wi_ext[:, :])
        wo0_s = consts.tile([P, KC], F32)
        nc.sync.dma_start(out=wo0_s, in_=Wo_ext[0:P, :])
        wo1_s = consts.tile([P, KC], F32)
        nc.sync.dma_start(out=wo1_s, in_=Wo_ext[P:H, :])
        bi0_s = consts.tile([P, 1], F32)
        nc.sync.dma_start(out=bi0_s, in_=bi_ext[0:P, :])
        bi1_s = consts.tile([P, 1], F32)
        nc.sync.dma_start(out=bi1_s, in_=bi_ext[P:H, :])
        bo_s = consts.tile([P, KC], F32)
        nc.sync.dma_start(out=bo_s, in_=bo_ext[0:1, :].partition_broadcast(P))
        id_s = consts.tile([P, P], F32)
        nc.sync.dma_start(out=id_s, in_=id_ext[:, :])

        x_tiles = []
        s_tiles = []
        row_cnt = []

        # ---- main pass: S per tile + stream A once ----
        with ExitStack() as mctx:
            cpool = mctx.enter_context(
                tc.tile_pool(name="cpool", bufs=1, space="PSUM"))
            wp = mctx.enter_context(
                tc.tile_pool(name="wp", bufs=2, space="PSUM"))

            cp_tiles = []
            for c in range(nch):
                w = min(MMF, n - c * MMF)
                cpt = cpool.tile([KA, w], F32, name=f"cp{c}", tag=f"cp{c}")
                cp_tiles.append(cpt)

            probe_s = outp.tile([1, nt], F32, name="probe_s")
            for t in range(nt):
                r0 = t * P
                rows = min(P, n - r0)
                x_t = xpool.tile([P, F], F32, name="x_t", tag="x_t")
                nc.sync.dma_start(out=x_t[:rows, :], in_=X_ext[r0:r0 + rows, :])
                a_t = apool.tile([P, n], F32, name="a_t", tag="a_t")
                nc.sync.dma_start(out=a_t[:rows, :], in_=A_ext[r0:r0 + rows, :])
                if stage == 0:
                    # DMA-floor measurement: consume one element per tile so
                    # the loads aren't dead, then skip all compute.
                    nc.vector.tensor_copy(out=probe_s[0:1, t:t + 1],
                                          in_=a_t[0:1, 0:1])
                    continue
                # rounded fp32r/bf16 copy of the A tile (PE full-rate matmul)
                if use_round:
                    a_r = apool.tile([P, n], MMDT, name="a_r", tag="a_r",
                                     bufs=rbufs)
                    eng = nc.gpsimd if round_engine == "gpsimd" else nc.vector
                    eng.tensor_copy(out=a_r[:rows, :], in_=a_t[:rows, :])
                else:
                    a_r = a_t

                # X_t^T via PE transpose
                xt_p = wp.tile([P, P], F32, name="xt_p", tag="w")
                nc.tensor.transpose(
                    out=xt_p[:, :rows], in_=x_t[:rows, :],
                    identity=id_s[:rows, :rows])
                xt_s = work.tile([P, P], F32, name="xt_s", tag="xt_s")
                nc.vector.tensor_copy(out=xt_s[:, :rows], in_=xt_p[:, :rows])

                # Hid^T = relu(W_in^T X^T + b_in), h-partitioned in 2 chunks
                hidT = []
                for c in range(2):
                    hid_p = wp.tile([P, P], F32, name="hid_p", tag="w")
                    nc.tensor.matmul(
                        out=hid_p[:, :rows],
                        lhsT=wi_s[:, c * P:(c + 1) * P],
                        rhs=xt_s[:, :rows],
                        start=True, stop=True)
                    hid_s = work.tile([P, P], F32, name="hid_s", tag=f"hid{c}")
                    bi = bi0_s if c == 0 else bi1_s
                    # relu(x + b) fused on DVE, PSUM -> SBUF
                    nc.vector.tensor_scalar(
                        out=hid_s[:, :rows], in0=hid_p[:, :rows],
                        scalar1=bi, scalar2=0.0,
                        op0=ALU.add, op1=ALU.max)
                    hidT.append(hid_s)

                # logits (rows, KC)
                lg_p = wp.tile([P, KC], F32, name="lg_p", tag="w")
                for c in range(2):
                    nc.tensor.matmul(
                        out=lg_p[:rows, :],
                        lhsT=hidT[c][:, :rows],
                        rhs=(wo0_s if c == 0 else wo1_s),
                        start=(c == 0), stop=(c == 1))
                lb = work.tile([P, KC], F32, name="lb", tag="lb")
                nc.vector.tensor_add(out=lb[:rows, :], in0=lg_p[:rows, :],
                                     in1=bo_s[:rows, :])

                # softmax over KC (free axis)
                s_t = spool.tile([P, KA], F32, name="s_t", tag="s_t")
                nmax = work.tile([P, 1], F32, name="nmax", tag="nmax")
                nc.vector.tensor_reduce(
                    out=nmax[:rows], in_=lb[:rows, :],
                    axis=mybir.AxisListType.X, op=ALU.max, negate=True)
                sexp = work.tile([P, 1], F32, name="sexp", tag="sexp")
                nc.scalar.activation(
                    out=s_t[:rows, 0:KC], in_=lb[:rows, :], func=AF.Exp,
                    bias=nmax[:rows], scale=1.0, accum_out=sexp[:rows])
                rcp = work.tile([P, 1], F32, name="rcp", tag="rcp")
                nc.vector.reciprocal(out=rcp[:rows], in_=sexp[:rows])
                nc.vector.tensor_scalar_mul(s_t[:rows, 0:KC], s_t[:rows, 0:KC],
                                            rcp[:rows])
                # ssum = sum_k S^2 -> augmented column
                # (tensor_tensor_reduce crashes TRN2 here; use mul+reduce)
                ssq = work.tile([P, KC], F32, name="ssq", tag="ssq")
                nc.vector.tensor_mul(ssq[:rows, :], s_t[:rows, 0:KC],
                                     s_t[:rows, 0:KC])
                nc.vector.tensor_reduce(
                    out=s_t[:rows, KC:KA], in_=ssq[:rows, :],
                    axis=mybir.AxisListType.X, op=ALU.add)

                nc.sync.dma_start(out=S_ext[r0:r0 + rows, :],
                                  in_=s_t[:rows, 0:KC])

                # rounded copy of Saug for the streaming matmuls
                if use_round:
                    s_r = spool.tile([P, KA], MMDT, name="s_r", tag="s_r")
                    nc.vector.tensor_copy(out=s_r[:rows, :], in_=s_t[:rows, :])
                else:
                    s_r = s_t

                # C' += Saug_t^T @ A_t   (the big streaming matmuls, fp32r)
                if stage >= 2:
                    for c in range(nch):
                        c0 = c * MMF
                        w = min(MMF, n - c0)
                        nc.tensor.matmul(
                            out=cp_tiles[c][:, :],
                            lhsT=s_r[:rows, :],
                            rhs=a_r[:rows, c0:c0 + w],
                            start=(t == 0), stop=(t == nt - 1),
                            skip_group_check=True)

                x_tiles.append(x_t)
                s_tiles.append(s_t)
                row_cnt.append(rows)

            # C' -> SBUF, den
            cs = csb.tile([KA, n], F32)
            den_s = outp.tile([1, 1], F32)
            if stage >= 2:
                for c in range(nch):
                    c0 = c * MMF
                    w = min(MMF, n - c0)
                    nc.vector.tensor_copy(out=cs[:, c0:c0 + w], in_=cp_tiles[c])
                nc.vector.tensor_reduce(
                    out=den_s, in_=cs[KC:KA, :],
                    axis=mybir.AxisListType.X, op=ALU.add)
            else:
                nc.vector.memset(cs, 0.0)
                nc.vector.memset(den_s, 1.0)
            nc.sync.dma_start(out=den_ext[:, :], in_=den_s)

        # ---- epilogue: A_pooled = C @ S, X_pooled = S^T X ----
        with ExitStack() as ectx:
            ep = ectx.enter_context(
                tc.tile_pool(name="ep", bufs=2, space="PSUM"))
            accp = ectx.enter_context(
                tc.tile_pool(name="accp", bufs=1, space="PSUM"))
            ap_p = accp.tile([KC, KC], F32, name="ap_p", tag="ap_p")
            xp_p = accp.tile([KC, F], F32, name="xp_p", tag="xp_p")

            for t in range(nt if stage >= 3 else 0):
                r0 = t * P
                rows = row_cnt[t]
                ct_p = ep.tile([P, KC], F32, name="ct_p", tag="ct")
                nc.tensor.transpose(
                    out=ct_p[:rows, :], in_=cs[0:KC, r0:r0 + rows],
                    identity=id_s[0:KC, 0:KC])
                ct_s = work.tile([P, KC], F32, name="ct_s", tag="ct_s")
                nc.vector.tensor_copy(out=ct_s[:rows, :], in_=ct_p[:rows, :])
                nc.tensor.matmul(
                    out=ap_p, lhsT=ct_s[:rows, :],
                    rhs=s_tiles[t][:rows, 0:KC],
                    start=(t == 0), stop=(t == nt - 1),
                    skip_group_check=True)
                nc.tensor.matmul(
                    out=xp_p, lhsT=s_tiles[t][:rows, 0:KC],
                    rhs=x_tiles[t][:rows, :],
                    start=(t == 0), stop=(t == nt - 1),
                    skip_group_check=True)

            ap_s = outp.tile([KC, KC], F32)
            xp_s = outp.tile([KC, F], F32)
            if stage >= 3:
                nc.vector.tensor_copy(out=ap_s, in_=ap_p)
                nc.vector.tensor_copy(out=xp_s, in_=xp_p)
            else:
                nc.vector.memset(ap_s, 0.0)
                nc.vector.memset(xp_s, 0.0)
            nc.sync.dma_start(out=APo_ext[:, :], in_=ap_s)
            nc.sync.dma_start(out=XP_ext[:, :], in_=xp_s)

    if not nc.is_finalized():
        nc.finalize()
    return nc


def TileCtx(nc):
    return tile.TileContext(nc)


_built = {}


def _get_nc(n=N):
    use_f32r = not bool(os.environ.get("MINCUT_NO_F32R"))
    stage = int(os.environ.get("MINCUT_STAGE", "3"))
    key = (n, use_f32r, stage)
    if key not in _built:
        _built[key] = build_nc(n, use_f32r=use_f32r, stage=stage)
    return _built[key]


LAST_RESULTS = None


def kernel(X, A, kernel_in, bias_in, kernel_out, bias_out):
    global LAST_RESULTS
    X = np.asarray(X, dtype=np.float32)
    A = np.asarray(A, dtype=np.float32)
    kernel_in = np.asarray(kernel_in, dtype=np.float32)
    bias_in = np.asarray(bias_in, dtype=np.float32)
    kernel_out = np.asarray(kernel_out, dtype=np.float32)
    bias_out = np.asarray(bias_out, dtype=np.float32)

    n = X.shape[1]
    nc = _get_nc(n)
    ident = np.eye(P, dtype=np.float32)
    in_maps = [{
        "A": np.ascontiguousarray(A[b]),
        "X": np.ascontiguousarray(X[b]),
        "W_in": kernel_in,
        "b_in": bias_in.reshape(H, 1),
        "W_out": kernel_out,
        "b_out": bias_out.reshape(1, KC),
        "ident": ident,
    } for b in range(B)]

    trace = bool(os.environ.get("MINCUT_TRACE"))
    LAST_RESULTS = run_bass_kernel_spmd(
        nc, in_maps, list(range(B)), trace=trace)
    res = LAST_RESULTS.results

    S = np.stack([res[b]["S_out"] for b in range(B)]).astype(np.float32)
    X_pooled = np.stack([res[b]["XP"] for b in range(B)]).astype(np.float32)
    APr = np.stack([res[b]["AP_raw"] for b in range(B)]).astype(np.float64)
    den = np.array([float(np.asarray(res[b]["den"]).reshape(-1)[0])
                    for b in range(B)])

    num = np.trace(APr, axis1=1, axis2=2)
    cut_loss = np.float32(np.mean(-(num / den)))
    ortho_loss = cut_loss

    kc = APr.shape[-1]
    Ap0 = APr * (1.0 - np.eye(kc))
    Dp = np.sqrt(Ap0.sum(-1)) + EPS          # (B, kc)
    A_norm = (Ap0 / Dp[:, None, :] / Dp[:, :, None]).astype(np.float32)

    return X_pooled, A_norm, S, cut_loss, ortho_loss
